# revision 2
# baseline (speedup 1.0000x reference)
"""ChebFCN2D Trainium2 kernel.

out[n, :] = W @ R(n) + b, where R(n) = [1, outer(Tx[1:32], Ty[1:32])] are
Chebyshev-product features of x[n, 0], x[n, 1]  (N = 262144, feat = 962).

Strategy (pure data parallel over 8 cores, 32768 samples/core):
  - basis via T_i(x) = cos(i * arccos x) = sin(2pi * (psi - round(psi))),
    psi = i*arccos(x)/pi + 1/4, evaluated by the ACT engine's Sin
  - psi comes from a tiny PE matmul (phase weights + 0.25 carrier row);
    round(psi) is an ACT copy to int32 (exact round-to-nearest-even);
    the centered fraction is one DVE subtract -> Sin stays in [-pi, pi]
  - the X basis is replicated 3x across heads by an identity matmul on the
    fraction, so a single fat Sin emits the SBUF operand of the H-product
  - W-contraction (mm1) and head-reduction (mm2) are small PE matmuls packed
    by partition-block tile positions; bias and the ones-feature ride
    constant-carrier rows so PSUM holds the finished transposed output
"""

import os
import sys

if "/opt/trn_rl_repo" not in sys.path:
    sys.path.insert(0, "/opt/trn_rl_repo")

from contextlib import ExitStack

import numpy as np

import concourse.bass as bass
import concourse.tile as tile
from concourse import bacc, mybir
from concourse.bass_utils import run_bass_kernel_spmd

F32 = mybir.dt.float32
I32 = mybir.dt.int32
BF16 = mybir.dt.bfloat16

N_CORES = 8
N = 262144
S = N // N_CORES          # samples per core = 32768
T = 512                   # samples per tile
NT = S // T               # 64 tiles
NSG = 4                   # super-groups of 4 groups of 4 tiles
TWO_PI = float(2.0 * np.pi)
USE_BF16_H = False        # bf16 product path (2x DVE) - set after accuracy test
KSTAGE = int(os.environ.get("KSTAGE", "99"))  # ablation: 1=phase/RR 2=+Sins 3=+mm1 4=+prod 5=+mm2 99=all

_CACHE = {}


def _host_consts(W, b):
    W = np.asarray(W, np.float32)
    b = np.asarray(b, np.float32)
    inv_pi = np.float32(1.0 / np.pi)
    iw = np.arange(1, 32, dtype=np.float32) * inv_pi

    wmm1 = np.zeros((32, 96), np.float32)
    for k in range(3):
        wmm1[:31, 32 * k:32 * k + 31] = W[k, 1:].reshape(31, 31).T  # [j, i]
        wmm1[31, 32 * k + 31] = 1.0  # ones-row carrier through mm1
    wmm1 = np.tile(wmm1, (4, 1))     # [128, 96] co-located per rhs block

    xid3 = np.zeros((32, 96), np.float32)
    for k in range(3):
        for i in range(31):
            xid3[i, 32 * k + i] = 1.0
        xid3[31, 32 * k + 31] = 1.0  # pad lanes carry t2=0.25 -> basis 1
    xid3 = np.tile(xid3, (4, 1))     # [128, 96]

    hdt = np.float32 if not USE_BF16_H else np.dtype("bfloat16")
    bias = b + W[:, 0]
    wmm2 = np.zeros((96, 3), np.float32)
    for k in range(3):
        for i in range(31):
            wmm2[32 * k + i, k] = 1.0
    if USE_BF16_H:
        import ml_dtypes
        hi = bias.astype(ml_dtypes.bfloat16).astype(np.float32)
        lo = (bias - hi).astype(ml_dtypes.bfloat16).astype(np.float32)
        wmm2[31, :] = hi
        wmm2[63, :] = lo
        wmm2 = wmm2.astype(ml_dtypes.bfloat16)
    else:
        wmm2[95, :] = bias

    wrep = np.zeros((5, 128), np.float32)
    for blk in range(4):
        wrep[blk, 32 * blk:32 * blk + 31] = iw
    wrep[4, :] = 1.0                 # rhs row4 = 0.25 -> +0.25 everywhere
    c025 = np.full((1, 16 * T), 0.25, np.float32)
    return {"wmm1": wmm1, "xid3": xid3, "wmm2": wmm2, "wrep": wrep,
            "c025": c025}


def _build_program():
    nc = bacc.Bacc("TRN2", target_bir_lowering=False, debug=False,
                   num_devices=N_CORES)
    hdt = BF16 if USE_BF16_H else F32
    d_xc = nc.dram_tensor("xc", [128, T], F32, kind="ExternalInput").ap()
    d_wmm1 = nc.dram_tensor("wmm1", [128, 96], F32, kind="ExternalInput").ap()
    d_xid3 = nc.dram_tensor("xid3", [128, 96], F32, kind="ExternalInput").ap()
    d_wmm2 = nc.dram_tensor("wmm2", [96, 3], hdt, kind="ExternalInput").ap()
    d_wrep = nc.dram_tensor("wrep", [5, 128], F32, kind="ExternalInput").ap()
    d_c025 = nc.dram_tensor("c025", [1, 16 * T], F32, kind="ExternalInput").ap()
    d_out = nc.dram_tensor("outT", [3, S], F32, kind="ExternalOutput").ap()

    with tile.TileContext(nc) as tc, ExitStack() as ctx:
        consts = ctx.enter_context(tc.tile_pool(name="consts", bufs=1))
        prep = ctx.enter_context(tc.tile_pool(name="prep", bufs=1))
        t2p = ctx.enter_context(tc.tile_pool(name="t2p", bufs=2))
        byp = ctx.enter_context(tc.tile_pool(name="byp", bufs=2))
        x3p = ctx.enter_context(tc.tile_pool(name="x3p", bufs=4))
        rrp = ctx.enter_context(tc.tile_pool(name="rrp", bufs=3))
        hp = ctx.enter_context(tc.tile_pool(name="h", bufs=3))
        ocp = ctx.enter_context(tc.tile_pool(name="ocp", bufs=2))
        ppsi = ctx.enter_context(tc.tile_pool(name="ppsi", bufs=2,
                                              space="PSUM"))
        ppx = ctx.enter_context(tc.tile_pool(name="ppx", bufs=2, space="PSUM"))
        pg = ctx.enter_context(tc.tile_pool(name="pg", bufs=1, space="PSUM"))
        pob = ctx.enter_context(tc.tile_pool(name="pob", bufs=1, space="PSUM"))

        t_wmm1 = consts.tile([128, 96], F32)
        nc.sync.dma_start(t_wmm1[:], d_wmm1)
        t_xid3 = consts.tile([128, 96], F32)
        nc.sync.dma_start(t_xid3[:], d_xid3)
        t_wmm2 = consts.tile([96, 3], hdt)
        nc.sync.dma_start(t_wmm2[:], d_wmm2)
        t_wrep = consts.tile([5, 128], F32)
        nc.sync.dma_start(t_wrep[:], d_wrep)

        # ---- prep: phi = arctan(sqrt((1-x)/(1+x))) on [128, 512] ----
        t_xc = prep.tile([128, T], F32)
        nc.sync.dma_start(t_xc[:], d_xc)
        t_c = prep.tile([128, T], F32)
        nc.vector.tensor_scalar_add(t_c[:], t_xc[:], 1.0)
        t_r = prep.tile([128, T], F32)
        nc.vector.reciprocal(t_r[:], t_c[:])
        t_d = prep.tile([128, T], F32)
        nc.vector.scalar_tensor_tensor(
            t_d[:], t_xc[:], 1.0, t_r[:],
            mybir.AluOpType.subtract, mybir.AluOpType.mult,
        )
        t_e = prep.tile([128, T], F32)
        nc.scalar.activation(t_e[:], t_d[:], mybir.ActivationFunctionType.Sqrt,
                             scale=-1.0)
        t_ph = prep.tile([128, T], F32)
        nc.scalar.activation(t_ph[:], t_e[:],
                             mybir.ActivationFunctionType.Arctan)

        # ---- compact [5, 16T]: tile t = 16b+g at row b, free g*T;
        #      row 4 = 0.25 ----
        t_thx = consts.tile([5, 16 * T], F32)
        t_thy = consts.tile([5, 16 * T], F32)
        nc.sync.dma_start(t_thx[0:4, :], t_ph[0:64, :])
        nc.sync.dma_start(t_thy[0:4, :], t_ph[64:128, :])
        nc.sync.dma_start(t_thx[4:5, :], d_c025)
        nc.sync.dma_start(t_thy[4:5, :], d_c025)

        for sg in range(NSG):
            # phase + centered fraction for the 4 groups of this super-group
            t2x = t2p.tile([128, 4 * T], F32, tag="t2x")
            t2y = t2p.tile([128, 4 * T], F32, tag="t2y")
            for gq in range(4):
                g = 4 * sg + gq
                for (thc, t2) in ((t_thx, t2x), (t_thy, t2y)):
                    psi = ppsi.tile([128, T], F32)
                    nc.tensor.matmul(psi[:], t_wrep[:],
                                     thc[:, g * T:(g + 1) * T],
                                     start=True, stop=True)
                    rnd = rrp.tile([128, T], I32)
                    nc.scalar.activation(rnd[:], psi[:],
                                         mybir.ActivationFunctionType.Copy)
                    nc.vector.tensor_tensor(
                        t2[:, gq * T:(gq + 1) * T], psi[:], rnd[:],
                        mybir.AluOpType.subtract)

            if KSTAGE < 2:
                continue
            by = byp.tile([128, 4 * T], F32)
            nc.scalar.activation(by[:], t2y[:],
                                 mybir.ActivationFunctionType.Sin,
                                 scale=TWO_PI)

            x3s = []
            for blk in range(4):
                x3 = x3p.tile([96, 4 * T], F32)
                for half in range(2):
                    px3 = ppx.tile([96, 2 * T], F32)
                    for q in range(2):
                        gq = 2 * half + q
                        nc.tensor.matmul(
                            px3[:, q * T:(q + 1) * T],
                            t_xid3[32 * blk:32 * blk + 32, :],
                            t2x[32 * blk:32 * blk + 32, gq * T:(gq + 1) * T],
                            start=True, stop=True,
                            tile_position=(32 * blk, 0))
                    nc.scalar.activation(
                        x3[:, 2 * half * T:2 * (half + 1) * T], px3[:],
                        mybir.ActivationFunctionType.Sin, scale=TWO_PI)
                x3s.append(x3)

            if KSTAGE < 3:
                continue
            for gq in range(4):
                g = 4 * sg + gq
                ob = pob.tile([99, T], F32)
                for blk in range(4):
                    yblk = by[32 * blk:32 * blk + 32, gq * T:(gq + 1) * T]
                    gt = pg.tile([96, T], F32)
                    nc.tensor.matmul(gt[:],
                                     t_wmm1[32 * blk:32 * blk + 32, :],
                                     yblk, start=True, stop=True,
                                     tile_position=(32 * blk, 0))
                    if KSTAGE < 4:
                        continue
                    h = hp.tile([96, T], hdt)
                    nc.vector.tensor_tensor(
                        h[:], gt[:],
                        x3s[blk][:, gq * T:(gq + 1) * T],
                        mybir.AluOpType.mult)
                    if KSTAGE >= 5:
                        nc.tensor.matmul(ob[32 * blk:32 * blk + 3, :],
                                         t_wmm2[:], h[:],
                                         start=True, stop=True,
                                         tile_position=(0, 32 * blk))
                if KSTAGE >= 5:
                    outc = ocp.tile([99, T], F32)
                    nc.vector.tensor_copy(outc[:], ob[:])
                    for blk in range(4):
                        t = 16 * blk + g
                        nc.sync.dma_start(d_out[:, t * T:(t + 1) * T],
                                          outc[32 * blk:32 * blk + 3, :])
    nc.compile()
    return nc


def _core_inputs(x, c, consts):
    xs = x[c * S:(c + 1) * S]                          # [S, 2]
    xc = np.empty((128, T), np.float32)
    xc[0:64] = xs[:, 0].reshape(64, T)
    xc[64:128] = xs[:, 1].reshape(64, T)
    return {"xc": xc, **consts}


def kernel(x, W, b):
    x = np.ascontiguousarray(np.asarray(x, np.float32))
    consts = _host_consts(W, b)
    if "nc" not in _CACHE:
        _CACHE["nc"] = _build_program()
    nc = _CACHE["nc"]

    in_maps = [_core_inputs(x, c, consts) for c in range(N_CORES)]

    res = run_bass_kernel_spmd(nc, in_maps, list(range(N_CORES))).results
    out = np.empty((N, 3), np.float32)
    for c in range(N_CORES):
        out[c * S:(c + 1) * S] = res[c]["outT"].T
    return out



# revision 23
# speedup vs baseline: 1.5462x; 1.5462x over previous
"""ChebFCN2D Trainium2 kernel.

out[n, :] = W @ R(n) + b, where R(n) = [1, outer(Tx[1:32], Ty[1:32])] are
Chebyshev-product features of x[n, 0], x[n, 1]  (N = 262144, feat = 962).

Strategy (pure data parallel over 8 cores, 32768 samples/core):
  - basis via T_i(x) = cos(i * arccos x) = sin(2pi * (psi - round(psi))),
    psi = i*arccos(x)/pi + 1/4, evaluated by the ACT engine's Sin
  - psi comes from a tiny PE matmul (phase weights + 0.25 carrier row);
    round(psi) is an ACT copy to int32 (exact round-to-nearest-even);
    the centered fraction is one DVE subtract -> Sin stays in [-pi, pi]
  - the X basis is replicated 3x across heads by an identity matmul on the
    fraction, so a single fat Sin emits the SBUF operand of the H-product
  - W-contraction (mm1) and head-reduction (mm2) are small PE matmuls packed
    by partition-block tile positions; bias and the ones-feature ride
    constant-carrier rows so PSUM holds the finished transposed output
"""

import os
import sys

if "/opt/trn_rl_repo" not in sys.path:
    sys.path.insert(0, "/opt/trn_rl_repo")

from contextlib import ExitStack

import numpy as np

import concourse.bass as bass
import concourse.tile as tile
from concourse import bacc, mybir
from concourse.bass_utils import run_bass_kernel_spmd

F32 = mybir.dt.float32
F32R = mybir.dt.float32r
I32 = mybir.dt.int32
BF16 = mybir.dt.bfloat16

N_CORES = 8
N = 262144
S = N // N_CORES          # samples per core = 32768
T = 512                   # samples per tile
NT = S // T               # 64 tiles
NSG = 4                   # super-groups of 4 groups of 4 tiles
TWO_PI = float(2.0 * np.pi)
USE_BF16_H = True         # bf16 product path: bf16 H + bf16 mm2 weights
KSTAGE = int(os.environ.get("KSTAGE", "99"))  # ablation: 1=phase/RR 2=+Sins 3=+mm1 4=+prod 5=+mm2 99=all

_CACHE = {}


def _host_consts(W, b):
    W = np.asarray(W, np.float32)
    b = np.asarray(b, np.float32)
    inv_pi = np.float32(1.0 / np.pi)
    iw = np.arange(1, 32, dtype=np.float32) * inv_pi

    wmm1 = np.zeros((32, 96), np.float32)
    for k in range(3):
        wmm1[:31, 32 * k:32 * k + 31] = W[k, 1:].reshape(31, 31).T  # [j, i]
        wmm1[31, 32 * k + 31] = 1.0  # ones-row carrier through mm1
    import ml_dtypes
    wmm1 = np.tile(wmm1, (4, 1)).astype(ml_dtypes.bfloat16)  # [128, 96]

    xid3 = np.zeros((32, 96), np.float32)
    for k in range(3):
        for i in range(31):
            xid3[i, 32 * k + i] = 1.0
        xid3[31, 32 * k + 31] = 1.0  # pad lanes carry t2=0.25 -> basis 1
    xid3 = np.tile(xid3, (4, 1))     # [128, 96]

    hdt = np.float32 if not USE_BF16_H else np.dtype("bfloat16")
    bias = b + W[:, 0]
    wmm2 = np.zeros((96, 3), np.float32)
    for k in range(3):
        for i in range(31):
            wmm2[32 * k + i, k] = 1.0
    if USE_BF16_H:
        hi = bias.astype(ml_dtypes.bfloat16).astype(np.float32)
        lo = (bias - hi).astype(ml_dtypes.bfloat16).astype(np.float32)
        wmm2[31, :] = hi
        wmm2[63, :] = lo
        wmm2 = wmm2.astype(ml_dtypes.bfloat16)
    else:
        wmm2[95, :] = bias

    wrep = np.zeros((5, 128), np.float32)
    for blk in range(4):
        wrep[blk, 32 * blk:32 * blk + 31] = iw
    wrep[4, :] = 1.0                 # rhs row4 = 0.25 -> +0.25 everywhere
    c025 = np.full((1, 16 * T), 0.25, np.float32)
    return {"wmm1": wmm1, "xid3": xid3, "wmm2": wmm2, "wrep": wrep,
            "c025": c025}


def _build_program():
    nc = bacc.Bacc("TRN2", target_bir_lowering=False, debug=False,
                   num_devices=N_CORES)
    hdt = BF16 if USE_BF16_H else F32
    d_xc = nc.dram_tensor("xc", [128, T], F32, kind="ExternalInput").ap()
    d_wmm1 = nc.dram_tensor("wmm1", [128, 96], BF16, kind="ExternalInput").ap()
    d_xid3 = nc.dram_tensor("xid3", [128, 96], F32, kind="ExternalInput").ap()
    d_wmm2 = nc.dram_tensor("wmm2", [96, 3], hdt, kind="ExternalInput").ap()
    d_wrep = nc.dram_tensor("wrep", [5, 128], F32, kind="ExternalInput").ap()
    d_c025 = nc.dram_tensor("c025", [1, 16 * T], F32, kind="ExternalInput").ap()
    d_out = nc.dram_tensor("outT", [3, S], F32, kind="ExternalOutput").ap()

    with tile.TileContext(nc) as tc, ExitStack() as ctx:
        consts = ctx.enter_context(tc.tile_pool(name="consts", bufs=1))
        prep = ctx.enter_context(tc.tile_pool(name="prep", bufs=1))
        t2p = ctx.enter_context(tc.tile_pool(name="t2p", bufs=2))
        byp = ctx.enter_context(tc.tile_pool(name="byp", bufs=2))
        x3p = ctx.enter_context(tc.tile_pool(name="x3p", bufs=4))
        rrp = ctx.enter_context(tc.tile_pool(name="rrp", bufs=3))
        hp = ctx.enter_context(tc.tile_pool(name="h", bufs=3))
        ocp = ctx.enter_context(tc.tile_pool(name="ocp", bufs=2))
        ppsi = ctx.enter_context(tc.tile_pool(name="ppsi", bufs=2,
                                              space="PSUM"))
        ppx = ctx.enter_context(tc.tile_pool(name="ppx", bufs=2, space="PSUM"))
        pg = ctx.enter_context(tc.tile_pool(name="pg", bufs=1, space="PSUM"))
        pob = ctx.enter_context(tc.tile_pool(name="pob", bufs=1, space="PSUM"))

        t_wmm1 = consts.tile([128, 96], BF16)
        nc.sync.dma_start(t_wmm1[:], d_wmm1)
        t_xid3 = consts.tile([128, 96], F32)
        nc.sync.dma_start(t_xid3[:], d_xid3)
        t_wmm2 = consts.tile([96, 3], hdt)
        nc.sync.dma_start(t_wmm2[:], d_wmm2)
        t_wrep = consts.tile([5, 128], F32)
        nc.sync.dma_start(t_wrep[:], d_wrep)

        # ---- prep: phi = arctan(sqrt((1-x)/(1+x))) on [128, 512] ----
        t_xc = prep.tile([128, T], F32)
        nc.sync.dma_start(t_xc[:], d_xc)
        t_c = prep.tile([128, T], F32)
        nc.vector.tensor_scalar_add(t_c[:], t_xc[:], 1.0)
        t_r = prep.tile([128, T], F32)
        nc.vector.reciprocal(t_r[:], t_c[:])
        t_d = prep.tile([128, T], F32)
        nc.vector.scalar_tensor_tensor(
            t_d[:], t_xc[:], 1.0, t_r[:],
            mybir.AluOpType.subtract, mybir.AluOpType.mult,
        )
        t_e = prep.tile([128, T], F32)
        nc.scalar.activation(t_e[:], t_d[:], mybir.ActivationFunctionType.Sqrt,
                             scale=-1.0)
        t_ph = prep.tile([128, T], F32)
        nc.scalar.activation(t_ph[:], t_e[:],
                             mybir.ActivationFunctionType.Arctan)

        # ---- compact [5, 16T]: tile t = 16b+g at row b, free g*T;
        #      row 4 = 0.25 ----
        t_thx = consts.tile([5, 16 * T], F32)
        t_thy = consts.tile([5, 16 * T], F32)
        nc.sync.dma_start(t_thx[0:4, :], t_ph[0:64, :])
        nc.sync.dma_start(t_thy[0:4, :], t_ph[64:128, :])
        nc.sync.dma_start(t_thx[4:5, :], d_c025)
        nc.sync.dma_start(t_thy[4:5, :], d_c025)

        for sg in range(NSG):
            # phase + centered fraction for the 4 groups of this super-group
            t2x = t2p.tile([128, 4 * T], F32, tag="t2x")
            t2y = t2p.tile([128, 4 * T], F32, tag="t2y")
            for gq in range(4):
                g = 4 * sg + gq
                for (thc, t2) in ((t_thx, t2x), (t_thy, t2y)):
                    psi = ppsi.tile([128, T], F32)
                    nc.tensor.matmul(psi[:], t_wrep[:],
                                     thc[:, g * T:(g + 1) * T],
                                     start=True, stop=True)
                    rnd = rrp.tile([128, T], I32)
                    nc.scalar.activation(rnd[:], psi[:],
                                         mybir.ActivationFunctionType.Copy)
                    nc.vector.tensor_tensor(
                        t2[:, gq * T:(gq + 1) * T], psi[:], rnd[:],
                        mybir.AluOpType.subtract)

            if KSTAGE < 2:
                continue
            by = byp.tile([128, 4 * T], BF16)
            nc.scalar.activation(by[:], t2y[:],
                                 mybir.ActivationFunctionType.Sin,
                                 scale=TWO_PI)

            x3s = []
            for blk in range(4):
                x3 = x3p.tile([96, 4 * T], F32)
                for half in range(2):
                    px3 = ppx.tile([96, 2 * T], F32)
                    for q in range(2):
                        gq = 2 * half + q
                        nc.tensor.matmul(
                            px3[:, q * T:(q + 1) * T],
                            t_xid3[32 * blk:32 * blk + 32, :],
                            t2x[32 * blk:32 * blk + 32, gq * T:(gq + 1) * T],
                            start=True, stop=True,
                            tile_position=(32 * blk, 0))
                    nc.scalar.activation(
                        x3[:, 2 * half * T:2 * (half + 1) * T], px3[:],
                        mybir.ActivationFunctionType.Sin, scale=TWO_PI)
                x3s.append(x3)

            if KSTAGE < 3:
                continue
            for gq in range(4):
                g = 4 * sg + gq
                ob = pob.tile([99, T], F32)
                for blk in range(4):
                    yblk = by[32 * blk:32 * blk + 32, gq * T:(gq + 1) * T]
                    gt = pg.tile([96, T], F32)
                    nc.tensor.matmul(gt[:],
                                     t_wmm1[32 * blk:32 * blk + 32, :],
                                     yblk, start=True, stop=True,
                                     tile_position=(32 * blk, 0))
                    if KSTAGE < 4:
                        continue
                    h = hp.tile([96, T], hdt)
                    nc.vector.tensor_tensor(
                        h[:], gt[:],
                        x3s[blk][:, gq * T:(gq + 1) * T],
                        mybir.AluOpType.mult)
                    if KSTAGE >= 5:
                        nc.tensor.matmul(ob[32 * blk:32 * blk + 3, :],
                                         t_wmm2[:], h[:],
                                         start=True, stop=True,
                                         tile_position=(0, 32 * blk))
                if KSTAGE >= 5:
                    outc = ocp.tile([99, T], F32)
                    nc.scalar.copy(outc[:], ob[:])
                    for blk in range(4):
                        t = 16 * blk + g
                        nc.sync.dma_start(d_out[:, t * T:(t + 1) * T],
                                          outc[32 * blk:32 * blk + 3, :])
    nc.compile()
    return nc


def _core_inputs(x, c, consts):
    xs = x[c * S:(c + 1) * S]                          # [S, 2]
    xc = np.empty((128, T), np.float32)
    xc[0:64] = xs[:, 0].reshape(64, T)
    xc[64:128] = xs[:, 1].reshape(64, T)
    return {"xc": xc, **consts}


def kernel(x, W, b):
    x = np.ascontiguousarray(np.asarray(x, np.float32))
    consts = _host_consts(W, b)
    if "nc" not in _CACHE:
        _CACHE["nc"] = _build_program()
    nc = _CACHE["nc"]

    in_maps = [_core_inputs(x, c, consts) for c in range(N_CORES)]

    res = run_bass_kernel_spmd(nc, in_maps, list(range(N_CORES))).results
    out = np.empty((N, 3), np.float32)
    for c in range(N_CORES):
        out[c * S:(c + 1) * S] = res[c]["outT"].T
    return out

